# revision 2
# baseline (speedup 1.0000x reference)
"""Trainium2 distributed kernel for the ACSConv Chebyshev graph conv.

Math (reference): with Tx0 = tile(x, (8,1)) [16384,16],
    Tx_{k} = 2*Ls@Tx_{k-1} - Tx_{k-2}   (Tx1 = Ls@Tx0)
    out = sum_k proj(Tx_k, W_k) + bias,  proj mixes the 8 angle blocks.

Distribution (8 NeuronCores): Ls row-sharded into contiguous 2048-row
blocks (= angle blocks). Core i holds LsT_i = Ls[2048i:2048(i+1), :].T
(host pre-transposed, bf16) and computes its Tx block each iteration;
blocks are AllGather'd between iterations. The projection splits per
angle slice of W, partial outputs are summed on host.

Per-core per-iteration matmul: Y.T[16, 2048] = sum_g TxTile_g.T @ LsT_g
with TxTile [128,16] stationary and LsT tiles [128, 512] moving, so the
1 GiB Ls matrix streams in natural (pre-transposed) layout at full DMA
bandwidth. Recurrence/projection/accumulation stay in fp32; only the
Ls stream and the stationary Tx tiles are bf16 (sim rel-err ~6e-3).
"""

import numpy as np
import ml_dtypes

import concourse.bass as bass
import concourse.mybir as mybir
import concourse.tile as tile
from concourse import bacc
from concourse.bass_utils import run_bass_kernel_spmd
from concourse.masks import make_identity

NCORES = 8
N = 2048          # nodes
CIN = 16
COUT = 32
NANG = 8          # angles
K = 15            # Chebyshev order
NATOT = NANG * N  # 16384
RPC = NATOT // NCORES   # rows of Ls per core = 2048
G = NATOT // 128        # 128 contraction k-tiles
MCH = RPC // 512        # 4 output m-chunks of 512

BF16 = mybir.dt.bfloat16
F32 = mybir.dt.float32
NP_BF16 = ml_dtypes.bfloat16

LS_BUFS = 12      # SBUF ls-stream prefetch depth (x 512KB)

_NC_CACHE = {}


def _build():
    nc = bacc.Bacc("TRN2", target_bir_lowering=False, debug=False,
                   num_devices=NCORES)

    lst = nc.dram_tensor("lst", [NATOT, RPC], BF16, kind="ExternalInput")
    xb = nc.dram_tensor("xb", [N, CIN], BF16, kind="ExternalInput")
    xt = nc.dram_tensor("xt", [CIN, N], F32, kind="ExternalInput")
    w = nc.dram_tensor("w", [K, CIN, COUT], F32, kind="ExternalInput")
    out = nc.dram_tensor("out", [COUT, RPC], F32, kind="ExternalOutput")

    lst_r = lst.ap().rearrange("(g p) c -> p g c", p=128)      # [128, G, RPC]

    with tile.TileContext(nc) as tc:
        with (
            tc.tile_pool(name="ls", bufs=LS_BUFS) as ls_pool,
            tc.tile_pool(name="tx", bufs=2) as tx_pool,
            tc.tile_pool(name="zt", bufs=3) as zt_pool,
            tc.tile_pool(name="znat", bufs=2) as znat_pool,
            tc.tile_pool(name="small", bufs=1) as small,
            tc.tile_pool(name="yps", bufs=4, space="PSUM") as yps,
            tc.tile_pool(name="pps", bufs=2, space="PSUM") as pps,
            tc.tile_pool(name="tps", bufs=2, space="PSUM") as tps,
            tc.tile_pool(name="dram", bufs=2, space="DRAM") as dram,
        ):
            # ---- preamble: constants and k=0 projection ----
            ident = small.tile([CIN, CIN], F32)
            make_identity(nc, ident[:])

            xb_sb = small.tile([128, N // 128, CIN], BF16)
            nc.sync.dma_start(xb_sb[:], xb.ap().rearrange("(t p) c -> p t c", p=128))

            xt_sb = small.tile([CIN, N], F32)
            nc.sync.dma_start(xt_sb[:], xt[:])

            w_sb = small.tile([CIN, K, COUT], F32)
            nc.sync.dma_start(w_sb[:], w.ap().rearrange("k p c -> p k c"))

            acc = small.tile([COUT, RPC], F32)
            for j in range(MCH):
                pj = pps.tile([COUT, 512], F32, name="pj", tag="proj")
                nc.tensor.matmul(pj[:], w_sb[:, 0, :], xt_sb[:, j * 512:(j + 1) * 512],
                                 start=True, stop=True)
                nc.vector.tensor_copy(acc[:, j * 512:(j + 1) * 512], pj[:])

            # ---- Chebyshev iterations k = 1..14 ----
            zts = {0: xt_sb}
            for k in range(1, K):
                # Y.T = (Ls_i @ Tx_{k-1}).T accumulated over G k-tiles
                ys = [yps.tile([CIN, 512], F32, name=f"y{j}", tag="y") for j in range(MCH)]
                if k == 1:
                    def lhs(g):
                        return xb_sb[:, g % (N // 128), :]
                else:
                    txk = tx_tiles  # noqa: F821  (set at end of prev iter)

                    def lhs(g):
                        return txk[:, g, :]
                for g in range(G):
                    ls_t = ls_pool.tile([128, RPC], BF16, name="ls_t", tag="ls")
                    nc.sync.dma_start(ls_t[:], lst_r[:, g, :])
                    for j in range(MCH):
                        nc.tensor.matmul(ys[j][:], lhs(g),
                                         ls_t[:, j * 512:(j + 1) * 512],
                                         start=(g == 0), stop=(g == G - 1))

                # recurrence in fp32: z_k = 2*Y - z_{k-2}  (z_1 = Y)
                zt = zt_pool.tile([CIN, RPC], F32, name="zt", tag="zt")
                for j in range(MCH):
                    dst = zt[:, j * 512:(j + 1) * 512]
                    if k == 1:
                        nc.vector.tensor_copy(dst, ys[j][:])
                    else:
                        nc.vector.scalar_tensor_tensor(
                            dst, ys[j][:], 2.0,
                            zts[k - 2][:, j * 512:(j + 1) * 512],
                            mybir.AluOpType.mult, mybir.AluOpType.subtract)
                zts[k] = zt

                # projection: acc += W_k[angle_i].T @ z_k.T
                for j in range(MCH):
                    pj = pps.tile([COUT, 512], F32, name="pj", tag="proj")
                    nc.tensor.matmul(pj[:], w_sb[:, k, :],
                                     zt[:, j * 512:(j + 1) * 512],
                                     start=True, stop=True)
                    nc.vector.tensor_tensor(acc[:, j * 512:(j + 1) * 512],
                                            acc[:, j * 512:(j + 1) * 512],
                                            pj[:], mybir.AluOpType.add)

                if k == K - 1:
                    break

                # transpose z_k -> natural [2048,16] bf16 and AllGather
                znat = znat_pool.tile([128, N // 128, CIN], BF16, name="znat", tag="znat")
                for c in range(4):
                    tr = tps.tile([128, 4 * CIN], F32, name="tr", tag="tr")
                    for t in range(4):
                        u = 4 * c + t
                        nc.tensor.transpose(tr[:, t * CIN:(t + 1) * CIN],
                                            zt[:, u * 128:(u + 1) * 128],
                                            ident[:])
                    nc.vector.tensor_copy(znat[:, 4 * c:4 * c + 4, :], tr[:])

                ag_in = dram.tile([N, CIN], BF16, name="ag_in", tag="agin")
                ag_out = dram.tile([NATOT, CIN], BF16, name="ag_out", tag="agout",
                                   addr_space="Shared")
                nc.sync.dma_start(ag_in.rearrange("(t p) c -> p t c", p=128),
                                  znat[:])
                nc.gpsimd.collective_compute(
                    "AllGather", mybir.AluOpType.bypass,
                    replica_groups=[list(range(NCORES))],
                    ins=[ag_in.opt()], outs=[ag_out.opt()])

                tx_tiles = tx_pool.tile([128, G, CIN], BF16, name="tx_tiles", tag="tx")
                nc.sync.dma_start(tx_tiles[:],
                                  ag_out.rearrange("(g p) c -> p g c", p=128))

            nc.sync.dma_start(out[:], acc[:])

    nc.compile()
    return nc


def _get_nc():
    if "nc" not in _NC_CACHE:
        _NC_CACHE["nc"] = _build()
    return _NC_CACHE["nc"]


def _shard(x, Ls, weight):
    in_maps = []
    xb = x.astype(NP_BF16)
    xtr = np.ascontiguousarray(x.T.astype(np.float32))
    for i in range(NCORES):
        lst_i = Ls[i * RPC:(i + 1) * RPC, :].T.astype(NP_BF16)
        w_i = np.ascontiguousarray(weight[:, i * CIN:(i + 1) * CIN, :])
        in_maps.append({"lst": lst_i, "xb": xb, "xt": xtr, "w": w_i})
    return in_maps


def run(x, Ls, weight, bias, trace=False, **kw):
    nc = _get_nc()
    in_maps = _shard(np.asarray(x), np.asarray(Ls), np.asarray(weight))
    res = run_bass_kernel_spmd(nc, in_maps, core_ids=list(range(NCORES)),
                               trace=trace, **kw)
    accs = [res.results[i]["out"] for i in range(NCORES)]
    full = np.sum(accs, axis=0, dtype=np.float32).T + np.asarray(bias)[None, :]
    return full.astype(np.float32), res


def kernel(x, Ls, weight, bias):
    out, _ = run(x, Ls, weight, bias, trace=False)
    return out


# revision 3
# speedup vs baseline: 1.1250x; 1.1250x over previous
"""Trainium2 distributed kernel for the ACSConv Chebyshev graph conv.

Math (reference): with Tx0 = tile(x, (8,1)) [16384,16],
    Tx_{k} = 2*Ls@Tx_{k-1} - Tx_{k-2}   (Tx1 = Ls@Tx0)
    out = sum_k proj(Tx_k, W_k) + bias,  proj mixes the 8 angle blocks.

Distribution (8 NeuronCores): Ls row-sharded into contiguous 2048-row
blocks (= angle blocks). Core i holds LsT_i = Ls[2048i:2048(i+1), :].T
(host pre-transposed, bf16) and computes its Tx block each iteration;
blocks are AllGather'd between iterations (payload in partition-major
[128,256] layout so both collective-adjacent DMAs use fat descriptors).
The projection splits per angle slice of W; partial outputs summed on
host.

Per-core per-iteration matmul: Y.T[16, 2048] = sum_g TxTile_g.T @ LsT_g
with TxTile [128,16] stationary and LsT tiles [128, 512] moving. The
last RES_KT k-tiles of LsT stay resident in SBUF across all 14
iterations (loaded once); the rest stream via 2 MiB DMAs alternating
between the Sync and Scalar HWDGE rings. Recurrence/projection/
accumulation stay in fp32; only the Ls stream and the stationary Tx
tiles are bf16 (sim rel-err ~6e-3, gate 2e-2).
"""

import numpy as np
import ml_dtypes

import concourse.bass as bass
import concourse.mybir as mybir
import concourse.tile as tile
from concourse import bacc
from concourse.bass_utils import run_bass_kernel_spmd
from concourse.masks import make_identity

NCORES = 8
N = 2048          # nodes
CIN = 16
COUT = 32
NANG = 8          # angles
K = 15            # Chebyshev order
NATOT = NANG * N  # 16384
RPC = NATOT // NCORES   # rows of Ls per core = 2048
G = NATOT // 128        # 128 contraction k-tiles
MCH = RPC // 512        # 4 output m-chunks of 512
TPG = N // 128          # 16 row-tiles per core block

BF16 = mybir.dt.bfloat16
F32 = mybir.dt.float32
NP_BF16 = ml_dtypes.bfloat16

RES_KT = 20       # k-tiles resident in SBUF (last RES_KT of the 128)
STREAM_KT = G - RES_KT
DMA_KT = 4        # k-tiles per streaming DMA (2 MiB)
LS_BUFS = 3       # streaming prefetch depth (x DMA_KT tiles)

_NC_CACHE = {}


def _build():
    nc = bacc.Bacc("TRN2", target_bir_lowering=False, debug=False,
                   num_devices=NCORES)

    lst = nc.dram_tensor("lst", [NATOT, RPC], BF16, kind="ExternalInput")
    xb = nc.dram_tensor("xb", [N, CIN], BF16, kind="ExternalInput")
    xt = nc.dram_tensor("xt", [CIN, N], F32, kind="ExternalInput")
    w = nc.dram_tensor("w", [K, CIN, COUT], F32, kind="ExternalInput")
    out = nc.dram_tensor("out", [COUT, RPC], F32, kind="ExternalOutput")

    lst_r = lst.ap().rearrange("(g p) c -> p g c", p=128)      # [128, G, RPC]

    with tile.TileContext(nc) as tc:
        with (
            tc.tile_pool(name="ls", bufs=LS_BUFS) as ls_pool,
            tc.tile_pool(name="tx", bufs=2) as tx_pool,
            tc.tile_pool(name="zt", bufs=3) as zt_pool,
            tc.tile_pool(name="znat", bufs=2) as znat_pool,
            tc.tile_pool(name="small", bufs=1) as small,
            tc.tile_pool(name="yps", bufs=4, space="PSUM") as yps,
            tc.tile_pool(name="pps", bufs=2, space="PSUM") as pps,
            tc.tile_pool(name="tps", bufs=2, space="PSUM") as tps,
            tc.tile_pool(name="dram", bufs=2, space="DRAM") as dram,
        ):
            # ---- preamble: constants, resident Ls tiles, k=0 projection ----
            ident = small.tile([CIN, CIN], F32)
            make_identity(nc, ident[:])

            xb_sb = small.tile([128, TPG, CIN], BF16)
            nc.gpsimd.dma_start(xb_sb[:], xb.ap().rearrange("(t p) c -> p t c", p=128))

            w_sb = small.tile([CIN, K, COUT], F32)
            nc.gpsimd.dma_start(w_sb[:], w.ap().rearrange("k p c -> p k c"))

            ls_res = small.tile([128, RES_KT, RPC], BF16)
            for r in range(0, RES_KT, DMA_KT):
                nc.gpsimd.dma_start(ls_res[:, r:r + DMA_KT, :],
                                    lst_r[:, STREAM_KT + r:STREAM_KT + r + DMA_KT, :])

            # zts[0] = x.T in fp32 (Tx0 block transposed), lives in the zt pool
            xt_sb = zt_pool.tile([CIN, N], F32, name="xt_sb", tag="zt")
            nc.gpsimd.dma_start(xt_sb[:], xt[:])

            acc = small.tile([COUT, RPC], F32)
            for j in range(MCH):
                pj = pps.tile([COUT, 512], F32, name="pj", tag="proj")
                nc.tensor.matmul(pj[:], w_sb[:, 0, :], xt_sb[:, j * 512:(j + 1) * 512],
                                 start=True, stop=True)
                nc.vector.tensor_copy(acc[:, j * 512:(j + 1) * 512], pj[:])

            # ---- Chebyshev iterations k = 1..14 ----
            zts = {0: xt_sb}
            for k in range(1, K):
                # Y.T = (Ls_i @ Tx_{k-1}).T accumulated over G k-tiles
                ys = [yps.tile([CIN, 512], F32, name=f"y{j}", tag="y")
                      for j in range(MCH)]
                if k == 1:
                    def lhs(g):
                        return xb_sb[:, g % TPG, :]
                else:
                    txk = tx_tiles  # noqa: F821  (set at end of prev iter)

                    def lhs(g):
                        return txk[:, g // TPG, g % TPG, :]
                ls_t = None
                for g in range(G):
                    if g < STREAM_KT:
                        if g % DMA_KT == 0:
                            ls_t = ls_pool.tile([128, DMA_KT, RPC], BF16,
                                                name="ls_t", tag="ls")
                            eng = nc.sync if (g // DMA_KT) % 2 == 0 else nc.scalar
                            eng.dma_start(ls_t[:], lst_r[:, g:g + DMA_KT, :])
                        src = ls_t[:, g % DMA_KT, :]
                    else:
                        src = ls_res[:, g - STREAM_KT, :]
                    for j in range(MCH):
                        nc.tensor.matmul(ys[j][:], lhs(g),
                                         src[:, j * 512:(j + 1) * 512],
                                         start=(g == 0), stop=(g == G - 1))

                # recurrence in fp32 (z_k = 2Y - z_{k-2}; z_1 = Y), then
                # transpose each 512-chunk to natural bf16 layout right away
                zt = zt_pool.tile([CIN, RPC], F32, name="zt", tag="zt")
                last = k == K - 1
                znat = None
                if not last:
                    znat = znat_pool.tile([128, TPG, CIN], BF16,
                                          name="znat", tag="znat")
                for j in range(MCH):
                    dst = zt[:, j * 512:(j + 1) * 512]
                    if k == 1:
                        nc.vector.tensor_copy(dst, ys[j][:])
                    else:
                        nc.vector.scalar_tensor_tensor(
                            dst, ys[j][:], 2.0,
                            zts[k - 2][:, j * 512:(j + 1) * 512],
                            mybir.AluOpType.mult, mybir.AluOpType.subtract)
                    if not last:
                        tr = tps.tile([128, 4 * CIN], F32, name="tr", tag="tr")
                        for t in range(4):
                            u = 4 * j + t
                            nc.tensor.transpose(tr[:, t * CIN:(t + 1) * CIN],
                                                zt[:, u * 128:(u + 1) * 128],
                                                ident[:])
                        nc.vector.tensor_copy(znat[:, 4 * j:4 * j + 4, :], tr[:])
                zts[k] = zt

                if not last:
                    # AllGather the block in partition-major [128, TPG*CIN]
                    # layout: rank r's rows land at ag_out[r*128:(r+1)*128].
                    ag_in = dram.tile([128, TPG * CIN], BF16, name="ag_in",
                                      tag="agin")
                    ag_out = dram.tile([NCORES * 128, TPG * CIN], BF16,
                                       name="ag_out", tag="agout",
                                       addr_space="Shared")
                    nc.gpsimd.dma_start(ag_in[:], znat[:])
                    nc.gpsimd.collective_compute(
                        "AllGather", mybir.AluOpType.bypass,
                        replica_groups=[list(range(NCORES))],
                        ins=[ag_in.opt()], outs=[ag_out.opt()])
                    tx_tiles = tx_pool.tile([128, NCORES, TPG, CIN], BF16,
                                            name="tx_tiles", tag="tx")
                    nc.gpsimd.dma_start(
                        tx_tiles[:],
                        ag_out.rearrange("(r p) w -> p r w", p=128))

                # projection (off the AG critical path): acc += W_k_i.T @ z.T
                for j in range(MCH):
                    pj = pps.tile([COUT, 512], F32, name="pj", tag="proj")
                    nc.tensor.matmul(pj[:], w_sb[:, k, :],
                                     zt[:, j * 512:(j + 1) * 512],
                                     start=True, stop=True)
                    nc.vector.tensor_tensor(acc[:, j * 512:(j + 1) * 512],
                                            acc[:, j * 512:(j + 1) * 512],
                                            pj[:], mybir.AluOpType.add)

            nc.sync.dma_start(out[:], acc[:])

    nc.compile()
    return nc


def _get_nc():
    if "nc" not in _NC_CACHE:
        _NC_CACHE["nc"] = _build()
    return _NC_CACHE["nc"]


def _shard(x, Ls, weight):
    in_maps = []
    xb = x.astype(NP_BF16)
    xtr = np.ascontiguousarray(x.T.astype(np.float32))
    for i in range(NCORES):
        lst_i = Ls[i * RPC:(i + 1) * RPC, :].T.astype(NP_BF16)
        w_i = np.ascontiguousarray(weight[:, i * CIN:(i + 1) * CIN, :])
        in_maps.append({"lst": lst_i, "xb": xb, "xt": xtr, "w": w_i})
    return in_maps


def run(x, Ls, weight, bias, trace=False, **kw):
    nc = _get_nc()
    in_maps = _shard(np.asarray(x), np.asarray(Ls), np.asarray(weight))
    res = run_bass_kernel_spmd(nc, in_maps, core_ids=list(range(NCORES)),
                               trace=trace, **kw)
    accs = [res.results[i]["out"] for i in range(NCORES)]
    full = np.sum(accs, axis=0, dtype=np.float32).T + np.asarray(bias)[None, :]
    return full.astype(np.float32), res


def kernel(x, Ls, weight, bias):
    out, _ = run(x, Ls, weight, bias, trace=False)
    return out
